# revision 36
# baseline (speedup 1.0000x reference)
"""Self-contained TRN2 Bass kernel for nn_MultiHeadAttn_91010357002583.

Multi-head attention (B=4, S=2048, D=1024, H=16, hd=64), eval mode,
mask all-ones, char_ids/seq_len unused by the reference.

Sharding: 8 cores = 4 batches x 2 query-row halves. Each core receives
x^T (bf16) for its batch with ITS query half's rows first, computes Q^T
for its half, and K^T/V for the batch. Scheme v2+exchange:
  - per (head-pair, q-half, k-tile) step both heads' score chunks land
    packed [A|B] in one [128,1024] PSUM tile (2 banks, double buffered)
    and get ONE exp activation (scalar engine is the attention-phase
    co-bottleneck);
  - softmax denominator rides as a 65th all-ones V column through the
    AV matmul (no separate ones-matmuls); normalization = reciprocal +
    gpsimd partition_broadcast + vector multiply, deferred off the
    (hp,qh) boundary;
  - head-groups 2,3 compute only their OWN K/V half and pairwise
    AllGather with the sibling core (rank-ordered full K/V DMA'd back —
    row order is irrelevant since softmax/AV sum over k; group 0,1 stay
    local so the collectives pipeline without stalling the PE);
  - QKV/fc projection chains use 1-bank PSUM (2 bufs), JIT-paced into
    the attention stream; fc for the first q-half overlaps group 3.
Output is a pure concatenation across cores.
"""

import math
import sys
from contextlib import ExitStack

import numpy as np
import ml_dtypes

for _p in ("/opt/trn_rl_repo", "/root/.axon_site/_ro/trn_rl_repo"):
    if _p not in sys.path:
        sys.path.insert(0, _p)

import concourse.bass as bass  # noqa: E402
import concourse.tile as tile  # noqa: E402
from concourse import bacc, mybir  # noqa: E402
from concourse.bass_utils import run_bass_kernel_spmd  # noqa: E402

bf16 = ml_dtypes.bfloat16
FP32 = mybir.dt.float32
BF16 = mybir.dt.bfloat16
AF = mybir.ActivationFunctionType

B, S, D, H = 4, 2048, 1024, 16
HD = D // H
SCALE = math.sqrt(HD)


class Cfg:
    def __init__(self, R=2048, Q=1024, Hn=16, D=1024, repeats=1,
                 attn_scheme="coltile", interleave=True, no_ones=False,
                 norm_mode="direct", abufs=3, dbufs=1,
                 xt_chunks=True, jit_prologue=True, kv_exchange=False):
        self.xt_chunks = xt_chunks
        self.jit_prologue = jit_prologue
        self.kv_exchange = kv_exchange
        assert R % 128 == 0 and Q % 128 == 0 and Hn % 4 == 0
        self.R, self.Q, self.Hn, self.D = R, Q, Hn, D
        self.FT = D // 128          # feature tiles (proj contraction)
        self.NCT = Hn // 2          # coltiles for Q (and K) = heads/2
        self.NRT = R // 128         # k row tiles
        self.NG = Hn // 4           # head groups of 4
        self.NJ = Hn * 64 // 128    # d-tiles for fc contraction
        self.NQT = Q // 128
        self.NRC = max(1, R // 1024)
        self.repeats = repeats
        self.attn_scheme = attn_scheme
        self.interleave = interleave
        self.no_ones = no_ones
        self.norm_mode = norm_mode
        self.abufs = abufs
        self.dbufs = dbufs
        self.scale = 1.0 / math.sqrt(64.0)


def build_nc(cfg: Cfg, num_devices=8):
    R, Q, Hn, Dm, FT = cfg.R, cfg.Q, cfg.Hn, cfg.D, cfg.FT
    nc = bacc.Bacc("TRN2", target_bir_lowering=False, debug=False,
                   enable_asserts=False, num_devices=num_devices)
    xt_d = nc.dram_tensor("xt", [Dm, R], BF16, kind="ExternalInput").ap()
    wqk_d = nc.dram_tensor("wqk", [Hn, 128, FT, 128], BF16,
                           kind="ExternalInput").ap()
    wv_d = nc.dram_tensor("wv", [FT, 128, Hn * 64], BF16,
                          kind="ExternalInput").ap()
    wfc_d = nc.dram_tensor("wfc", [cfg.NJ, 128, Dm], BF16,
                           kind="ExternalInput").ap()
    bfc_d = nc.dram_tensor("bfc", [128, Dm], FP32, kind="ExternalInput").ap()
    y_d = nc.dram_tensor("y", [Q, Dm], FP32, kind="ExternalOutput").ap()
    body = build_body_v2 if cfg.attn_scheme == "v2" else build_body
    with tile.TileContext(nc) as tc:
        with ExitStack() as ctx:
            body(ctx, tc, cfg, xt_d, wqk_d, wv_d, wfc_d, bfc_d, y_d)
    nc.finalize()
    return nc


def build_body_v2(ctx, tc, cfg: Cfg, xt_d, wqk_d, wv_d, wfc_d, bfc_d, y_d):
    """Scheme v2: fused denominator (65th V column), per-step packed
    [headA|headB] score tiles with a single exp activation, 1-bank PSUM
    accumulators, 1-bank projection chains.

    PSUM: S pool 2x[128,1024] (4 banks) + o pool 2x[65,512] (2 banks)
          + p pool 2x[128,512] (2 banks) = 8 banks.
    """
    nc = tc.nc
    R, Q, Hn, Dm, FT = cfg.R, cfg.Q, cfg.Hn, cfg.D, cfg.FT
    NCT, NRT, NG, NJ, NQT = cfg.NCT, cfg.NRT, cfg.NG, cfg.NJ, cfg.NQT

    persist = ctx.enter_context(tc.tile_pool(name="persist", bufs=1))
    wqk_pool = ctx.enter_context(
        tc.tile_pool(name="wqk", bufs=8 if cfg.kv_exchange else 6))
    wv_pool = ctx.enter_context(tc.tile_pool(name="wv", bufs=2))
    attn_pool = ctx.enter_context(tc.tile_pool(name="attn", bufs=cfg.abufs))
    den_pool = ctx.enter_context(tc.tile_pool(name="den", bufs=2))
    ysb_pool = ctx.enter_context(tc.tile_pool(name="ysb", bufs=2))
    Spool = ctx.enter_context(tc.tile_pool(name="ps_S", bufs=2, space="PSUM"))
    opool = ctx.enter_context(tc.tile_pool(name="ps_o", bufs=2, space="PSUM"))
    ppool = ctx.enter_context(tc.tile_pool(name="ps_p", bufs=2, space="PSUM"))
    if cfg.kv_exchange:
        stg_pool = ctx.enter_context(tc.tile_pool(name="stg", bufs=2))
        dr_pool = ctx.enter_context(
            tc.tile_pool(name="dr", bufs=2, space="DRAM"))

    for _rep in range(cfg.repeats):
        xt_sb = persist.tile([128, FT, R], BF16, tag="xt")
        KT_sb = persist.tile([128, NCT, R], BF16, tag="kt")
        QT_sb = persist.tile([128, NCT, Q], BF16, tag="qt")
        V_sb = persist.tile([128, NRT, Hn, 65], BF16, tag="v")
        OT_sb = persist.tile([128, NJ, Q], BF16, tag="ot")
        wfc_sb = persist.tile([128, NJ, Dm], BF16, tag="wfc")
        bfc_sb = persist.tile([128, Dm], FP32, tag="bfc")
        ones1_sb = persist.tile([1, 64], BF16, tag="ones1")
        nc.vector.memset(ones1_sb[:], 1.0)
        nc.vector.memset(V_sb[:, :, :, 64:65], 1.0)
        # preload the exp table set during the DMA-bound prologue
        warm = den_pool.tile([1, 16], FP32, tag="warm", name="warm")
        nc.vector.memset(warm[:], 0.0)
        nc.scalar.activation(warm[:], warm[:], AF.Exp)

        # first 512 columns of xt only — the rest streams in after the
        # prefix units have queued their (gating) weight DMAs
        def dma_xt(rc, fts=None):
            for ft in (range(FT) if fts is None else fts):
                if cfg.xt_chunks:
                    nc.sync.dma_start(
                        xt_sb[:, ft, rc * 512:(rc + 1) * 512],
                        xt_d[ft * 128:(ft + 1) * 128, rc * 512:(rc + 1) * 512])
                elif rc == 0:
                    nc.sync.dma_start(
                        xt_sb[:, ft, :], xt_d[ft * 128:(ft + 1) * 128, :])
        dma_xt(0)

        wqk_tiles = {}

        def load_wqk(j):
            t = wqk_pool.tile([128, FT, 128], BF16, tag="wqk", name="wqk_t")
            nc.sync.dma_start(t[:], wqk_d[j])
            wqk_tiles[j] = t

        def emit_qkv_units(g, exchange=False):
            """Closures emitting one 1-bank psum chain each of group g's
            QKV projection work. With exchange=True, each core computes only
            its own half of K/V (its xt columns 0:R/2) into a DRAM staging
            buffer, pairwise-AllGathers, and DMAs the rank-ordered full K/V
            back (row order is irrelevant: softmax and AV sum over k)."""
            if g >= NG:
                return []
            cts = [2 * g, 2 * g + 1]
            units = []

            state = {}

            def load_w(g=g, cts=cts):
                for ct in cts:
                    load_wqk(ct)
                    load_wqk(NCT + ct)
                wv_sb = wv_pool.tile([128, FT, 256], BF16, tag="wv",
                                     name="wv_t")
                nc.sync.dma_start(
                    wv_sb[:],
                    wv_d[:, :, g * 256:(g + 1) * 256].rearrange(
                        "f p c -> p f c"))
                return wv_sb

            def ensure_w():
                if "wv" not in state:
                    state["wv"] = load_w()

            def ensure_dr():
                if "din" not in state:
                    state["din"] = dr_pool.tile([128, 4096], BF16,
                                                tag="din", name="din")
                    state["dout"] = dr_pool.tile([256, 4096], BF16,
                                                 tag="dout", name="dout")

            def ensure_kw(ct):
                if ("kw", ct) not in state:
                    load_wqk(NCT + ct)
                    state[("kw", ct)] = True

            def ensure_qw(ct):
                if ("qw", ct) not in state:
                    load_wqk(ct)
                    state[("qw", ct)] = True

            def ensure_wv():
                if "wvx" not in state:
                    wv_sb = wv_pool.tile([128, FT, 256], BF16, tag="wv",
                                         name="wv_t")
                    nc.sync.dma_start(
                        wv_sb[:],
                        wv_d[:, :, g * 256:(g + 1) * 256].rearrange(
                            "f p c -> p f c"))
                    state["wvx"] = wv_sb

            def q_unit(ct, qc):
                def emit():
                    if exchange:
                        ensure_qw(ct)
                    else:
                        ensure_w()
                    ps = ppool.tile([128, 512], FP32, tag="p", name="ps_q")
                    for ft in range(FT):
                        nc.tensor.matmul(
                            ps, wqk_tiles[ct][:, ft, :],
                            xt_sb[:, ft, qc * 512:(qc + 1) * 512],
                            start=(ft == 0), stop=(ft == FT - 1))
                    nc.vector.tensor_copy(
                        QT_sb[:, ct, qc * 512:(qc + 1) * 512], ps)
                return emit

            def k_unit(ct, rc):
                ci = ct - 2 * g

                def emit():
                    if exchange:
                        ensure_dr()
                        ensure_kw(ct)
                    else:
                        ensure_w()
                    ps = ppool.tile([128, 512], FP32, tag="p", name="ps_k")
                    for ft in range(FT):
                        nc.tensor.matmul(
                            ps, wqk_tiles[NCT + ct][:, ft, :],
                            xt_sb[:, ft, rc * 512:(rc + 1) * 512],
                            start=(ft == 0), stop=(ft == FT - 1))
                    if exchange:
                        kst = stg_pool.tile([128, 512], BF16, tag="kst",
                                            name="kst")
                        nc.vector.tensor_copy(kst, ps)
                        nc.sync.dma_start(
                            state["din"][:, ci * 1024 + rc * 512:
                                         ci * 1024 + (rc + 1) * 512], kst[:])
                    else:
                        nc.vector.tensor_copy(
                            KT_sb[:, ct, rc * 512:(rc + 1) * 512], ps)
                return emit

            def v_unit(rt):
                def emit():
                    if exchange:
                        ensure_dr()
                        ensure_wv()
                    else:
                        ensure_w()
                    ps = ppool.tile([128, 512], FP32, tag="p",
                                    name="ps_v")[:, :256]
                    for ft in range(FT):
                        nc.tensor.matmul(
                            ps, xt_sb[:, ft, rt * 128:(rt + 1) * 128],
                            state["wvx" if exchange else "wv"][:, ft, :],
                            start=(ft == 0), stop=(ft == FT - 1))
                    if exchange:
                        vst = stg_pool.tile([128, 256], BF16, tag="vst",
                                            name="vst")
                        nc.vector.tensor_copy(vst, ps)
                        nc.sync.dma_start(
                            state["din"][:, 2048 + rt * 256:
                                         2048 + (rt + 1) * 256], vst[:])
                    else:
                        nc.vector.tensor_copy(
                            V_sb[:, rt, 4 * g:4 * g + 4, 0:64],
                            ps.rearrange("p (h c) -> p h c", c=64))
                return emit

            PAIRS = [[0, 1], [2, 3], [4, 5], [6, 7]]

            def cc_unit():
                def emit():
                    din, dout = state["din"], state["dout"]
                    nc.gpsimd.collective_compute(
                        "AllGather", mybir.AluOpType.bypass,
                        replica_groups=PAIRS, ins=[din[:]], outs=[dout[:]])
                    for s in range(2):
                        rows = dout[s * 128:(s + 1) * 128, :]
                        for ci, ct in enumerate(cts):
                            nc.sync.dma_start(
                                KT_sb[:, ct, s * 1024:(s + 1) * 1024],
                                rows[:, ci * 1024:(ci + 1) * 1024])
                        vsrc = rows[:, 2048:4096].rearrange(
                            "p (r h c) -> p r h c", h=4, c=64)
                        for h in range(4):
                            nc.sync.dma_start(
                                V_sb[:, s * 8:s * 8 + 8, 4 * g + h, 0:64],
                                vsrc[:, :, h, :])
                return emit

            if not exchange:
                for ct in cts:
                    for qc in range(Q // 512):
                        units.append(q_unit(ct, qc))
                    for rc in range(R // 512):
                        units.append(k_unit(ct, rc))
                for rt in range(NRT):
                    units.append(v_unit(rt))
                return units
            # exchange: K-half chains + K collective first, then V-half
            # chains + V collective, then the (local) Q chains
            kv = []
            for ct in cts:
                for rc in range(R // 1024):
                    kv.append(k_unit(ct, rc))
            for rt in range(NRT // 2):
                kv.append(v_unit(rt))
            kv.append(cc_unit())
            qs = []
            for ct in cts:
                for qc in range(Q // 512):
                    qs.append(q_unit(ct, qc))
            return kv, qs

        pending_fin = []

        def norm_copy(o_, hp, qh, half):
            """Copy the accumulator out of PSUM (frees the o tile fast) and
            queue the rest of the normalization as a deferred closure."""
            q0 = qh * 512
            st = den_pool.tile([65, 512], FP32, tag="st", name="st")
            nc.vector.tensor_copy(st, o_)

            def fin():
                rcp = den_pool.tile([1, 512], FP32, tag="rcp", name="rcp")
                nc.vector.reciprocal(rcp, st[64:65, :])
                bc = den_pool.tile([64, 512], FP32, tag="bc", name="bc")
                nc.gpsimd.partition_broadcast(bc[:], rcp[:], channels=64)
                nc.vector.tensor_mul(
                    OT_sb[half * 64:half * 64 + 64, hp, q0:q0 + 512],
                    st[0:64, :], bc)

            pending_fin.append(fin)

        def fc_unit(qt):
            def emit():
                yt = ysb_pool.tile([128, Dm], FP32, tag="y", name="yt")
                for cc in range(0, Dm, 512):
                    ps = ppool.tile([128, 512], FP32, tag="p", name="ps_fc")
                    for j in range(NJ):
                        nc.tensor.matmul(
                            ps, OT_sb[:, j, qt * 128:(qt + 1) * 128],
                            wfc_sb[:, j, cc:cc + 512],
                            start=(j == 0), stop=(j == NJ - 1))
                    nc.vector.tensor_add(yt[:, cc:cc + 512], ps,
                                         bfc_sb[:, cc:cc + 512])
                nc.sync.dma_start(y_d[qt * 128:(qt + 1) * 128, :], yt[:])
            return emit

        def attn_step(hp, qh, kt, oA, oB):
            q0 = qh * 512
            S = Spool.tile([128, 1024], FP32, tag="S", name="S")
            nc.tensor.matmul(
                S[:, 0:512], KT_sb[0:64, hp, kt * 128:(kt + 1) * 128],
                QT_sb[0:64, hp, q0:q0 + 512], start=True, stop=True)
            nc.tensor.matmul(
                S[:, 512:1024], KT_sb[64:128, hp, kt * 128:(kt + 1) * 128],
                QT_sb[64:128, hp, q0:q0 + 512], start=True, stop=True)
            a = attn_pool.tile([128, 1024], BF16, tag="aT", name="a")
            nc.scalar.activation(a[:], S, AF.Exp, scale=cfg.scale)
            st, sp = (kt == 0), (kt == NRT - 1)
            nc.tensor.matmul(oA[:], V_sb[:, kt, 2 * hp, :],
                             a[:, 0:512], start=st, stop=sp)
            nc.tensor.matmul(oB[:], V_sb[:, kt, 2 * hp + 1, :],
                             a[:, 512:1024], start=st, stop=sp)

        # ---- prologue: minimal group-0 prefix; rest interleaves into the
        # first attention steps (JIT per-dependency order) ----
        u0 = {}
        g0_units = emit_qkv_units(0)
        # emit_qkv_units order: ct0:[q0,q1,k0..k3], ct1:[q0,q1,k0..k3], v0..15
        u0["q00"], u0["q01"] = g0_units[0], g0_units[1]
        u0["k00"], u0["k01"], u0["k02"], u0["k03"] = g0_units[2:6]
        u0["q10"], u0["q11"] = g0_units[6], g0_units[7]
        u0["k10"], u0["k11"], u0["k12"], u0["k13"] = g0_units[8:12]
        vs = g0_units[12:]
        if cfg.jit_prologue:
            prefix = (u0["q00"], u0["k00"], vs[0])
            g0_stream = [vs[1], vs[2], vs[3], u0["k01"], vs[4], vs[5], vs[6],
                         u0["k02"], vs[7], vs[8], vs[9], u0["k03"], vs[10],
                         u0["q10"], u0["k10"], vs[11], vs[12], u0["k11"],
                         vs[13], vs[14], u0["k12"], vs[15], u0["k13"],
                         u0["q01"], u0["q11"]]
        else:
            prefix = tuple(g0_units)
            g0_stream = []
        for u in prefix:
            u()
        for rc in range(1, R // 512):
            dma_xt(rc)
        nc.sync.dma_start(wfc_sb[:], wfc_d.rearrange("j p d -> p j d"))
        nc.sync.dma_start(bfc_sb[:], bfc_d[:])

        fc_done = 0
        for g in range(NG):
            if g == 0:
                # g0's own remaining units JIT over the first 16 steps, then
                # group 1's units over the rest
                if cfg.kv_exchange:
                    # only group 3 exchanges K/V halves: its collective is
                    # staged from group 0 and lands with ~60us margin, so
                    # the PE never stalls on it (group 2's collective would
                    # finish right at its own deadline - net zero)
                    xunits = {3: emit_qkv_units(3, exchange=True)}
                    streams = [(list(g0_stream), 0, 16),
                               (emit_qkv_units(1), 16, 44),
                               (xunits[3][0], 44, 56)]
                else:
                    streams = [(list(g0_stream), 0, 16),
                               (emit_qkv_units(1), 16, 64)]
            elif cfg.kv_exchange:
                if g == 1:
                    streams = [(emit_qkv_units(2), 0, 48)]
                elif g == 2:
                    streams = [(xunits[3][1], 0, 48)]
                else:
                    streams = []
            else:
                streams = [(emit_qkv_units(g + 1), 0, 64)]
            steps = [(qh, hp, kt) for qh in range(Q // 512)
                     for hp in (2 * g, 2 * g + 1) for kt in range(NRT)]
            o_tiles = {}
            uis = [0] * len(streams)
            for si, (qh, hp, kt) in enumerate(steps):
                if kt == 0:
                    o_tiles[(hp, qh)] = (
                        opool.tile([65, 512], FP32, tag="o", name="oA"),
                        opool.tile([65, 512], FP32, tag="o", name="oB"))
                oA, oB = o_tiles[(hp, qh)]
                attn_step(hp, qh, kt, oA, oB)
                if kt == NRT - 1:
                    norm_copy(oA, hp, qh, 0)
                    norm_copy(oB, hp, qh, 1)
                elif pending_fin:
                    pending_fin.pop(0)()
                if cfg.interleave:
                    for si0, (stream, lo, hi) in enumerate(streams):
                        if si + 1 <= lo:
                            continue
                        want = min(len(stream),
                                   (si + 1 - lo) * len(stream) // (hi - lo))
                        while uis[si0] < want:
                            stream[uis[si0]]()
                            uis[si0] += 1
                # group 3 second q-half: interleave the first-half fc
                if g == NG - 1 and si >= 36 and (si - 36) % 8 == 0 \
                        and fc_done < NQT // 2 and not pending_fin:
                    fc_unit(fc_done)()
                    fc_done += 1
            for si0, (stream, lo, hi) in enumerate(streams):
                while uis[si0] < len(stream):
                    stream[uis[si0]]()
                    uis[si0] += 1
            if g < NG - 1:
                while pending_fin:
                    pending_fin.pop(0)()

        while pending_fin:
            pending_fin.pop(0)()
        for qt in range(fc_done, NQT):
            fc_unit(qt)()


def build_body(ctx, tc, cfg: Cfg, xt_d, wqk_d, wv_d, wfc_d, bfc_d, y_d):
    nc = tc.nc
    R, Q, Hn, Dm, FT = cfg.R, cfg.Q, cfg.Hn, cfg.D, cfg.FT
    NCT, NRT, NG, NJ, NQT, NRC = (cfg.NCT, cfg.NRT, cfg.NG, cfg.NJ,
                                  cfg.NQT, cfg.NRC)
    m65 = cfg.attn_scheme == "m65"

    persist = ctx.enter_context(tc.tile_pool(name="persist", bufs=1))
    wqk_pool = ctx.enter_context(tc.tile_pool(name="wqk", bufs=6))
    wv_pool = ctx.enter_context(tc.tile_pool(name="wv", bufs=2))
    attn_pool = ctx.enter_context(tc.tile_pool(name="attn", bufs=cfg.abufs))
    ysb_pool = ctx.enter_context(tc.tile_pool(name="ysb", bufs=2))
    den_pool = ctx.enter_context(tc.tile_pool(name="den", bufs=cfg.dbufs))
    spool = ctx.enter_context(tc.tile_pool(name="ps_s", bufs=2, space="PSUM"))
    opool = ctx.enter_context(tc.tile_pool(name="ps_o", bufs=2, space="PSUM"))

    for _rep in range(cfg.repeats):
        xt_sb = persist.tile([128, FT, R], BF16, tag="xt")
        KT_sb = persist.tile([128, NCT, R], BF16, tag="kt")
        QT_sb = persist.tile([128, NCT, Q], BF16, tag="qt")
        VW = 65 if m65 else 64
        V_sb = persist.tile([128, NRT, Hn, VW], BF16, tag="v")
        OT_sb = persist.tile([128, NJ, Q], BF16, tag="ot")
        wfc_sb = persist.tile([128, NJ, Dm], BF16, tag="wfc")
        bfc_sb = persist.tile([128, Dm], FP32, tag="bfc")
        if m65:
            ones1_sb = persist.tile([1, 64], BF16, tag="ones1")
            nc.vector.memset(ones1_sb[:], 1.0)
            nc.vector.memset(V_sb[:, :, :, 64:65], 1.0)
        else:
            ones_sb = persist.tile([128, 64], BF16, tag="ones")
            nc.vector.memset(ones_sb[:], 1.0)

        for ft in range(FT):
            nc.sync.dma_start(xt_sb[:, ft, :], xt_d[ft * 128:(ft + 1) * 128, :])

        wqk_tiles = {}

        def load_wqk(j):
            t = wqk_pool.tile([128, FT, 128], BF16, tag="wqk", name="wqk_t")
            nc.sync.dma_start(t[:], wqk_d[j])
            wqk_tiles[j] = t

        def emit_qkv_units(g):
            """Return a list of closures, each emitting one psum-chain of
            group g's QKV projection work."""
            cts = [2 * g, 2 * g + 1]
            units = []

            def load_w(g=g, cts=cts):
                for ct in cts:
                    load_wqk(ct)
                    load_wqk(NCT + ct)
                wv_sb = wv_pool.tile([128, FT, 256], BF16, tag="wv",
                                     name="wv_t")
                nc.sync.dma_start(
                    wv_sb[:],
                    wv_d[:, :, g * 256:(g + 1) * 256].rearrange(
                        "f p c -> p f c"))
                return wv_sb

            state = {}

            def ensure_w():
                if "wv" not in state:
                    state["wv"] = load_w()

            def ensure_dr():
                if "din" not in state:
                    state["din"] = dr_pool.tile([128, 4096], BF16,
                                                tag="din", name="din")
                    state["dout"] = dr_pool.tile([256, 4096], BF16,
                                                 tag="dout", name="dout")

            def ensure_kw(ct):
                if ("kw", ct) not in state:
                    load_wqk(NCT + ct)
                    state[("kw", ct)] = True

            def ensure_qw(ct):
                if ("qw", ct) not in state:
                    load_wqk(ct)
                    state[("qw", ct)] = True

            def ensure_wv():
                if "wvx" not in state:
                    wv_sb = wv_pool.tile([128, FT, 256], BF16, tag="wv",
                                         name="wv_t")
                    nc.sync.dma_start(
                        wv_sb[:],
                        wv_d[:, :, g * 256:(g + 1) * 256].rearrange(
                            "f p c -> p f c"))
                    state["wvx"] = wv_sb

            def q_unit(ct):
                def emit():
                    ensure_w()
                    ps = spool.tile([128, 1024], FP32, tag="ps_s",
                                    name="ps_q")[:, :Q]
                    for sc in range(0, Q, 512):
                        sn = min(512, Q - sc)
                        for ft in range(FT):
                            nc.tensor.matmul(
                                ps[:, sc:sc + sn], wqk_tiles[ct][:, ft, :],
                                xt_sb[:, ft, sc:sc + sn],
                                start=(ft == 0), stop=(ft == FT - 1))
                    nc.vector.tensor_copy(QT_sb[:, ct, :], ps)
                return emit

            def k_unit(ct, rc):
                def emit():
                    ensure_w()
                    rn = min(1024, R - rc * 1024)
                    ps = spool.tile([128, 1024], FP32, tag="ps_s",
                                    name="ps_k")[:, :rn]
                    for sc in range(0, rn, 512):
                        sn = min(512, rn - sc)
                        for ft in range(FT):
                            nc.tensor.matmul(
                                ps[:, sc:sc + sn],
                                wqk_tiles[NCT + ct][:, ft, :],
                                xt_sb[:, ft, rc * 1024 + sc:rc * 1024 + sc + sn],
                                start=(ft == 0), stop=(ft == FT - 1))
                    nc.vector.tensor_copy(
                        KT_sb[:, ct, rc * 1024:rc * 1024 + rn], ps)
                return emit

            def v_unit(rt):
                def emit():
                    ensure_w()
                    ps = spool.tile([128, 1024], FP32, tag="ps_s",
                                    name="ps_v")[:, :256]
                    for ft in range(FT):
                        nc.tensor.matmul(
                            ps, xt_sb[:, ft, rt * 128:(rt + 1) * 128],
                            state["wvx" if exchange else "wv"][:, ft, :],
                            start=(ft == 0), stop=(ft == FT - 1))
                    nc.vector.tensor_copy(
                        V_sb[:, rt, 4 * g:4 * g + 4, 0:64],
                        ps.rearrange("p (h c) -> p h c", c=64))
                return emit

            for ct in cts:
                units.append(q_unit(ct))
                units.append(k_unit(ct, 0))
                if NRC > 1:
                    units.append(k_unit(ct, 1))
            for rt in range(NRT):
                units.append(v_unit(rt))
            return units

        def norm_and_store(o_, hp, half):
            """divide numerator rows by the fused denominator, write OT."""
            if m65:
                # copy psum out immediately to free the accumulator slot
                st = den_pool.tile([65, 1024], FP32, tag="st",
                                   name="st")[:, :Q]
                nc.vector.tensor_copy(st, o_[0:65, :])
                rcp = den_pool.tile([1, 1024], FP32, tag="rcp",
                                    name="rcp")[:, :Q]
                nc.vector.reciprocal(rcp, st[64:65, :])
                # bf16 hi/lo split so the bf16 broadcast matmul is exact
                hi = den_pool.tile([1, 1024], BF16, tag="rhi",
                                   name="rhi")[:, :Q]
                lo = den_pool.tile([1, 1024], BF16, tag="rlo",
                                   name="rlo")[:, :Q]
                tmp = den_pool.tile([1, 1024], FP32, tag="rtmp",
                                    name="rtmp")[:, :Q]
                nc.vector.tensor_copy(hi, rcp)
                nc.vector.tensor_tensor(tmp, rcp, hi,
                                        mybir.AluOpType.subtract)
                nc.vector.tensor_copy(lo, tmp)
                bc = spool.tile([128, 1024], FP32, tag="ps_s",
                                name="bc")[0:64, :Q]
                for sc in range(0, Q, 512):
                    sn = min(512, Q - sc)
                    s_ = slice(sc, sc + sn)
                    nc.tensor.matmul(bc[:, s_], ones1_sb[:], hi[:, s_],
                                     start=True, stop=False)
                    nc.tensor.matmul(bc[:, s_], ones1_sb[:], lo[:, s_],
                                     start=False, stop=True)
                nc.vector.tensor_mul(OT_sb[half * 64:half * 64 + 64, hp, :],
                                     st[0:64, :], bc)
            elif cfg.no_ones:
                nc.vector.tensor_copy(OT_sb[half * 64:half * 64 + 64, hp, :],
                                      o_[0:64, :])
            elif cfg.norm_mode == "copyout":
                st = den_pool.tile([128, 1024], FP32, tag="stc",
                                   name="stc")[:, :Q]
                nc.vector.tensor_copy(st, o_[:, :])
                den = den_pool.tile([64, 1024], FP32, tag="den",
                                    name="den")[:, :Q]
                nc.vector.reciprocal(den, st[64:128, :])
                nc.vector.tensor_mul(OT_sb[half * 64:half * 64 + 64, hp, :],
                                     st[0:64, :], den)
            else:
                den = den_pool.tile([64, 1024], FP32, tag="den",
                                    name="den")[:, :Q]
                nc.vector.reciprocal(den, o_[64:128, :])
                nc.vector.tensor_mul(OT_sb[half * 64:half * 64 + 64, hp, :],
                                     o_[0:64, :], den)

        def attn_step(hp, kt, oA, oB):
            psA = spool.tile([128, 1024], FP32, tag="ps_s", name="psA")[:, :Q]
            psB = spool.tile([128, 1024], FP32, tag="ps_s", name="psB")[:, :Q]
            for sc in range(0, Q, 512):
                sn = min(512, Q - sc)
                nc.tensor.matmul(
                    psA[:, sc:sc + sn],
                    KT_sb[0:64, hp, kt * 128:(kt + 1) * 128],
                    QT_sb[0:64, hp, sc:sc + sn], start=True, stop=True)
                nc.tensor.matmul(
                    psB[:, sc:sc + sn],
                    KT_sb[64:128, hp, kt * 128:(kt + 1) * 128],
                    QT_sb[64:128, hp, sc:sc + sn], start=True, stop=True)
            aA = attn_pool.tile([128, Q], BF16, tag="aT", name="aA")
            aB = attn_pool.tile([128, Q], BF16, tag="aT", name="aB")
            nc.scalar.activation(aA[:], psA, AF.Exp, scale=cfg.scale)
            nc.scalar.activation(aB[:], psB, AF.Exp, scale=cfg.scale)
            st, sp = (kt == 0), (kt == NRT - 1)
            for sc in range(0, Q, 512):
                sn = min(512, Q - sc)
                s_ = slice(sc, sc + sn)
                if m65:
                    nc.tensor.matmul(oA[0:65, s_], V_sb[:, kt, 2 * hp, :],
                                     aA[:, s_], start=st, stop=sp)
                    nc.tensor.matmul(oB[0:65, s_], V_sb[:, kt, 2 * hp + 1, :],
                                     aB[:, s_], start=st, stop=sp)
                elif cfg.attn_scheme == "swap":
                    # complementary col-groups across tiles: V_A(grp0,oA) ||
                    # V_B(grp64,oB), then ones_A(grp64,oA) || ones_B(grp0,oB).
                    # Per-tile writer order identical to the safe layout.
                    nc.tensor.matmul(oA[0:64, s_],
                                     V_sb[:, kt, 2 * hp, 0:64],
                                     aA[:, s_], start=st, stop=sp)
                    nc.tensor.matmul(oB[64:128, s_],
                                     V_sb[:, kt, 2 * hp + 1, 0:64],
                                     aB[:, s_], start=st, stop=sp)
                    nc.tensor.matmul(oA[64:128, s_], ones_sb[:],
                                     aA[:, s_], start=st, stop=sp)
                    nc.tensor.matmul(oB[0:64, s_], ones_sb[:],
                                     aB[:, s_], start=st, stop=sp)
                else:
                    nc.tensor.matmul(oA[0:64, s_],
                                     V_sb[:, kt, 2 * hp, 0:64],
                                     aA[:, s_], start=st, stop=sp)
                    if not cfg.no_ones:
                        nc.tensor.matmul(oA[64:128, s_], ones_sb[:],
                                         aA[:, s_], start=st, stop=sp)
                    nc.tensor.matmul(oB[0:64, s_],
                                     V_sb[:, kt, 2 * hp + 1, 0:64],
                                     aB[:, s_], start=st, stop=sp)
                    if not cfg.no_ones:
                        nc.tensor.matmul(oB[64:128, s_], ones_sb[:],
                                         aB[:, s_], start=st, stop=sp)

        # ---- main loop: group attention with next group's QKV interleaved
        units = emit_qkv_units(0)
        for u in units:
            u()
        # fc weights aren't needed until the tail — keep them off the
        # critical-path DMA window at kernel start
        nc.sync.dma_start(wfc_sb[:], wfc_d.rearrange("j p d -> p j d"))
        nc.sync.dma_start(bfc_sb[:], bfc_d[:])
        for g in range(NG):
            next_units = emit_qkv_units(g + 1) if g + 1 < NG else []
            steps = [(hp, kt) for hp in (2 * g, 2 * g + 1)
                     for kt in range(NRT)]
            o_tiles = {}
            ui = 0
            for si, (hp, kt) in enumerate(steps):
                if kt == 0:
                    o_tiles[hp] = (
                        opool.tile([128, 1024], FP32, tag="ps_o",
                                   name="oA")[:, :Q],
                        opool.tile([128, 1024], FP32, tag="ps_o",
                                   name="oB")[:, :Q])
                oA, oB = o_tiles[hp]
                attn_step(hp, kt, oA, oB)
                if cfg.interleave:
                    want = (si + 1) * len(next_units) // len(steps)
                    while ui < want:
                        next_units[ui]()
                        ui += 1
                if kt == NRT - 1:
                    norm_and_store(oA, hp, 0)
                    if cfg.attn_scheme == "swap":
                        den = den_pool.tile([64, 1024], FP32, tag="den",
                                            name="denS")[:, :Q]
                        nc.vector.reciprocal(den, oB[0:64, :])
                        nc.vector.tensor_mul(OT_sb[64:128, hp, :],
                                             oB[64:128, :], den)
                    else:
                        norm_and_store(oB, hp, 1)
            while ui < len(next_units):
                next_units[ui]()
                ui += 1
            if not cfg.interleave:
                pass

        # ---- fc ----
        for qt in range(NQT):
            ps = spool.tile([128, 1024], FP32, tag="ps_s",
                            name="ps_fc")[:, :Dm]
            for cc in range(0, Dm, 512):
                for j in range(NJ):
                    nc.tensor.matmul(
                        ps[:, cc:cc + 512],
                        OT_sb[:, j, qt * 128:(qt + 1) * 128],
                        wfc_sb[:, j, cc:cc + 512],
                        start=(j == 0), stop=(j == NJ - 1))
            yt = ysb_pool.tile([128, Dm], FP32, tag="y", name="yt")
            nc.vector.tensor_add(yt[:], ps, bfc_sb[:])
            nc.sync.dma_start(y_d[qt * 128:(qt + 1) * 128, :], yt[:])


# ---------------- host side ----------------

def prep_core_inputs(cfg: Cfg, xb_perm, W_qkv, W_fc, b_fc):
    """xb_perm: [R, D] f32, rows already permuted (this core's q rows first)."""
    Dm, Hn, FT, NCT, NJ = cfg.D, cfg.Hn, cfg.FT, cfg.NCT, cfg.NJ
    xt = np.ascontiguousarray(xb_perm.T).astype(bf16)
    Wq = W_qkv[:, :NCT * 128]
    Wk = W_qkv[:, Dm:Dm + NCT * 128]
    Wv = W_qkv[:, 2 * Dm:2 * Dm + Hn * 64]
    wq_t = Wq.reshape(FT, 128, NCT, 128).transpose(2, 1, 0, 3)
    wk_t = Wk.reshape(FT, 128, NCT, 128).transpose(2, 1, 0, 3)
    wqk = np.ascontiguousarray(
        np.concatenate([wq_t, wk_t], axis=0)).astype(bf16)
    wv = np.ascontiguousarray(Wv.reshape(FT, 128, Hn * 64)).astype(bf16)
    wfc = np.ascontiguousarray(
        W_fc[:NJ * 128].reshape(NJ, 128, Dm)).astype(bf16)
    bfc = np.ascontiguousarray(
        np.broadcast_to(b_fc.astype(np.float32), (128, Dm)))
    return {"xt": xt, "wqk": wqk, "wv": wv, "wfc": wfc, "bfc": bfc}


_CACHE = {}


def _get_nc(repeats=1):
    key = ("nc", repeats)
    if key not in _CACHE:
        _CACHE[key] = build_nc(Cfg(R=S, Q=S // 2, Hn=H, D=D, repeats=repeats,
                                   attn_scheme="v2", kv_exchange=True))
    return _CACHE[key]


def make_in_maps(x, W_qkv, W_fc, b_fc):
    cfg = Cfg(R=S, Q=S // 2, Hn=H, D=D)
    x = np.asarray(x, dtype=np.float32)
    in_maps = []
    for c in range(8):
        b, half = divmod(c, 2)
        r0 = half * (S // 2)
        order = np.concatenate([
            np.arange(r0, r0 + S // 2),
            np.arange(0, r0),
            np.arange(r0 + S // 2, S),
        ])
        xb = x[b][order]
        in_maps.append(prep_core_inputs(
            cfg, xb, np.asarray(W_qkv, np.float32),
            np.asarray(W_fc, np.float32), np.asarray(b_fc, np.float32)))
    return in_maps


def kernel(x, char_ids, seq_len, mask, W_qkv, W_fc, b_fc):
    """Full inputs in, full [B, S, D] float32 output out."""
    import os
    # the axon NTFF trace hook is unavailable in this container; make sure
    # an inherited BASS_TRACE=1 cannot send us down that (crashing) path
    os.environ["BASS_NEVER_TRACE"] = "1"
    nc = _get_nc(repeats=1)
    in_maps = make_in_maps(x, W_qkv, W_fc, b_fc)
    res = run_bass_kernel_spmd(nc, in_maps, core_ids=list(range(8)))
    out = np.empty((B, S, D), dtype=np.float32)
    for c in range(8):
        b, half = divmod(c, 2)
        r0 = half * (S // 2)
        out[b, r0:r0 + S // 2, :] = res.results[c]["y"]
    return out

